# revision 31
# baseline (speedup 1.0000x reference)
"""Trainium2 Bass kernel for nn_ComposedFeatureTransformer (embedding lookup).

out_s[b, :] = bias + sum_k values_s[b, k] * merged_weight[indices_s[b, k], :]
for s in {0, 1}.

Strategy (data-parallel over batch, 512 rows/core, both feature sets), V2:

No table repack at all — rows are gathered directly from the original f32
table via the packetized SWDGE `dma_gather`, using a mod-8 alignment trick:

  Row v starts at byte 4128*v = 256-aligned only when 32*(v%8) == 0, but for
  a FIXED r = v%8 the element [4128*v - 32*r, +4352) IS 256-aligned:
  4128*(8q+r) - 32*r = 33024*q + 4096*r. So the flat table reshaped to
  [5632, 8256] f32 (8 rows per group) gives, per bucket r, a gather view
  w8[:, 1024*r : 1024*r + 1088] with elem 4352 B / stride 33024 B — both
  multiples of 256 — and the real row at constant f32 offset 8*r inside the
  element. Bucket indices are q = v>>3 < 5632, fitting int16 directly.

Per (set, 128-batch tile) cell, lookups are bucketed by v%8 on the host;
each bucket's slots are gathered in ONE dma_gather call (GROUP=8; fewer
calls measured ~1.9 us/call faster on HW), padded to 128-chunks with -1
indices whose descriptors the SWDGE drops — per-core valid counts are read
at runtime from SBUF into a Pool register, refunding most of the padding's
HBM traffic. The one-hot stationary matrices are built on-device (DVE
tensor_scalar: M[p,j] = (j==tgt[p])*val[p]) and the reduction runs as
PSUM-accumulated matmuls in f32r (full PE rate on the 512-wide moving
slices): out[128b, D] += M_chunk[128slots, 128b]^T @ G_chunk[128, D].
Bias is added during the single PSUM->SBUF drain per cell.

Host-side preprocessing only touches indices/values (4 MB of int data);
the 186 MB table is never touched on host.
"""

import numpy as np

import concourse.bacc as bacc
import concourse.bass as bass
import concourse.mybir as mybir
import concourse.tile as tile
from concourse.bass_utils import run_bass_kernel_spmd

N_CORES = 8
BATCH = 4096
PER_CORE = BATCH // N_CORES  # 512
K = 32
V = 45056
D = 1032
P = 128
R = 8  # mod-8 alignment buckets
Q = V // R  # 5632 bucket rows
ESTEP = 8256  # f32 elems per 8-row group (33024 B)
ELEM = 1088  # gather element f32 elems (4352 B)
N_TILES = PER_CORE // P  # 4 batch tiles per set
NST = 2 * N_TILES  # (set, tile) cells per core
GROUP = 8  # max chunks (of 128 rows) per dma_gather call (one call per
# (cell,bucket) at cb<=8 — fewer calls measured ~1.9 us/call faster on HW)

TRACE = False  # set by test harness to collect an NTFF profile
LAST_RESULT = None  # BassKernelResults of the last run (for profiling)
PAD_FULL = False  # diagnostic: pad with real idx-0 descriptors (full counts)
RING = 32768  # SWDGE descriptor ring bytes (16 B/desc)
NQUEUES = 4  # SWDGE queues (1..4); gather calls round-robin across them

_NC_CACHE = {}


def _groups(cb):
    """Split cb chunks into dma_gather calls of <= GROUP chunks."""
    out = []
    while cb > 0:
        g = min(GROUP, cb)
        out.append(g)
        cb -= g
    return out


def _build(cbs, reps=None):
    """Build the SPMD kernel, specialized on the per-(cell,bucket) chunk
    counts cbs (tuple of NST tuples of R ints). reps wraps the body in a
    hardware loop (timing probes only)."""
    import contextlib

    nc = bacc.Bacc(
        "TRN2",
        debug=False,
        num_devices=N_CORES,
        dynamic_dma_scratch_size=RING,
        num_swdge_queues=NQUEUES,
    )
    f32 = mybir.dt.float32
    f32r = mybir.dt.float32r
    i16 = mybir.dt.int16
    i32 = mybir.dt.int32

    nch = sum(sum(c) for c in cbs)  # total chunks per core
    cols = nch * 8  # idx columns (128/16 per chunk)
    ncalls = sum(len(_groups(cb)) for c in cbs for cb in c)

    w8_d = nc.dram_tensor("weight", [Q, ESTEP], f32r, kind="ExternalInput")
    gidx_d = nc.dram_tensor("gidx", [P, cols], i16, kind="ExternalInput")
    tgt_d = nc.dram_tensor("tgt", [P, nch], f32, kind="ExternalInput")
    val_d = nc.dram_tensor("val", [P, nch], f32, kind="ExternalInput")
    cidx_d = nc.dram_tensor("cidx", [P, P], f32, kind="ExternalInput")
    cnt_d = nc.dram_tensor("counts", [1, ncalls], i32, kind="ExternalInput")
    bias_d = nc.dram_tensor("bias_rep", [P, D], f32, kind="ExternalInput")
    out_d = [
        nc.dram_tensor(f"out{s}", [PER_CORE, D], f32, kind="ExternalOutput")
        for s in range(2)
    ]

    with tile.TileContext(nc) as tc:
        with (
            tc.tile_pool(name="const", bufs=1) as const_pool,
            tc.tile_pool(name="gat", bufs=1) as gat_pool,
            tc.tile_pool(name="mst", bufs=4) as mst_pool,
            tc.tile_pool(name="psum", bufs=2, space="PSUM") as psum_pool,
            tc.tile_pool(name="outp", bufs=2) as out_pool,
        ):
            bias_sb = const_pool.tile([P, D], f32, tag="bias")
            nc.sync.dma_start(out=bias_sb[:], in_=bias_d[:])
            idx_sb = const_pool.tile([P, cols], i16, tag="idx")
            nc.sync.dma_start(out=idx_sb[:], in_=gidx_d[:])
            cnt_sb = const_pool.tile([1, ncalls], i32, tag="cnt")
            nc.sync.dma_start(out=cnt_sb[:], in_=cnt_d[:])
            tgt_sb = const_pool.tile([P, nch], f32, tag="tgt")
            nc.sync.dma_start(out=tgt_sb[:], in_=tgt_d[:])
            val_sb = const_pool.tile([P, nch], f32, tag="val")
            nc.sync.dma_start(out=val_sb[:], in_=val_d[:])
            cidx_sb = const_pool.tile([P, P], f32, tag="cidx")
            nc.sync.dma_start(out=cidx_sb[:], in_=cidx_d[:])

            # Fixed gather buffers (manual 4-deep rotation, subtile deps do
            # the pipelining). -1-padded tail slots are never written by the
            # SWDGE, and stale SBUF x 0 in the matmul would still poison
            # PSUM if it were NaN — so _pack_inputs full-pads the first call
            # reaching each (buffer, chunk extent) with real idx-0
            # descriptors, guaranteeing every byte a matmul reads was
            # written by some gather. (A rotating pool can't express this —
            # buffer slots are scheduler-assigned.)
            gat_bufs = []
            for i in range(4):
                gb = gat_pool.tile(
                    [P, GROUP, ELEM], f32r, name=f"gatbuf{i}", tag=f"g{i}"
                )
                gat_bufs.append(gb)

            # rotation of count registers; reuse distance 16 exceeds the
            # Pool engine's 8-deep instruction queues, so a queued gather
            # has always latched its count before the register is reloaded
            cnt_regs = [
                nc.gpsimd.alloc_register(f"cntreg{i}") for i in range(16)
            ]

            loop_cm = tc.For_i(0, reps, 1) if reps else contextlib.nullcontext()
            with loop_cm:
                col = 0
                chg = 0
                cnt_i = 0
                gi = 0
                for st in range(NST):
                    s, t = divmod(st, N_TILES)
                    cell_chunks = sum(cbs[st])
                    pt = psum_pool.tile([P, D], f32, tag="ps")
                    done = 0
                    for r in range(R):
                        src = w8_d[:, 1024 * r : 1024 * r + ELEM]
                        for g in _groups(cbs[st][r]):
                            gt = gat_bufs[gi % 4]
                            gi += 1
                            cnt = cnt_regs[cnt_i % 16]
                            nc.gpsimd.reg_load(
                                cnt, cnt_sb[0:1, cnt_i : cnt_i + 1]
                            )
                            nc.gpsimd.dma_gather(
                                gt[:, 0:g, :],
                                src,
                                idx_sb[:, col : col + 8 * g],
                                g * P,
                                cnt,
                                ELEM,
                                elem_step=ESTEP,
                                queue_num=cnt_i % NQUEUES,
                            )
                            # build the one-hot stationary matrices on DVE:
                            # M[p, j] = (j == tgt[p, ch]) * val[p, ch]
                            mt = mst_pool.tile([P, GROUP * P], f32r, tag="m")
                            for c in range(g):
                                nc.vector.tensor_scalar(
                                    out=mt[:, c * P : (c + 1) * P],
                                    in0=cidx_sb[:],
                                    scalar1=tgt_sb[:, chg + c : chg + c + 1],
                                    scalar2=val_sb[:, chg + c : chg + c + 1],
                                    op0=mybir.AluOpType.is_equal,
                                    op1=mybir.AluOpType.mult,
                                )
                            for c in range(g):
                                lhsT = mt[:, c * P : (c + 1) * P]
                                first = done == 0
                                last = done == cell_chunks - 1
                                for lo, hi in ((0, 512), (512, 1024), (1024, D)):
                                    nc.tensor.matmul(
                                        pt[:, lo:hi],
                                        lhsT,
                                        gt[:, c, 8 * r + lo : 8 * r + hi],
                                        start=first,
                                        stop=last,
                                    )
                                done += 1
                            col += 8 * g
                            chg += g
                            cnt_i += 1
                    ot = out_pool.tile([P, D], f32, tag="o")
                    nc.vector.tensor_add(out=ot[:], in0=pt[:], in1=bias_sb[:])
                    nc.sync.dma_start(
                        out=out_d[s][t * P : (t + 1) * P, :], in_=ot[:]
                    )

    nc.compile()
    return nc


def _get_nc(cbs):
    if cbs not in _NC_CACHE:
        _NC_CACHE[cbs] = _build(cbs)
    return _NC_CACHE[cbs]


def _pack_inputs(idx, val):
    """idx/val: [2, BATCH, K] int64/f32 (full). Returns (cbs, per-core list
    of dicts with gidx/mmat/counts)."""
    # per (core, cell, bucket) slot lists
    slot_idx = {}
    slot_b = {}
    slot_v = {}
    counts = np.zeros((N_CORES, NST, R), dtype=np.int64)
    b_local = np.repeat(np.arange(P, dtype=np.int64), K)  # [128*K]
    for c in range(N_CORES):
        for st in range(NST):
            s, t = divmod(st, N_TILES)
            b0 = c * PER_CORE + t * P
            fi = idx[s, b0 : b0 + P, :].reshape(-1).astype(np.int64)
            fv = val[s, b0 : b0 + P, :].reshape(-1)
            for r in range(R):
                sel = (fi & 7) == r
                slot_idx[c, st, r] = (fi[sel] >> 3).astype(np.int16)
                slot_b[c, st, r] = b_local[sel]
                slot_v[c, st, r] = fv[sel]
                counts[c, st, r] = sel.sum()

    # compiled chunk counts: max over cores per (cell, bucket)
    mx = counts.max(axis=0)
    cb = np.maximum(1, -(-mx // P))
    cbs = tuple(tuple(int(x) for x in row) for row in cb)
    nch = int(cb.sum())
    npad = nch * P

    per_core = []
    for c in range(N_CORES):
        flat_idx = np.full(npad, -1, dtype=np.int16)
        flat_b = np.zeros(npad, dtype=np.int64)
        flat_v = np.zeros(npad, dtype=np.float32)
        call_cnt = []
        off = 0
        gi = 0  # mirrors _build's gather-buffer rotation
        buf_extent = [0, 0, 0, 0]  # max chunk extent written per buffer
        for st in range(NST):
            for r in range(R):
                n = int(counts[c, st, r])
                flat_idx[off : off + n] = slot_idx[c, st, r]
                flat_b[off : off + n] = slot_b[c, st, r]
                flat_v[off : off + n] = slot_v[c, st, r]
                # per-call valid counts; ensure >= 1 per call (inject a
                # harmless idx-0/val-0 slot into empty calls). The first
                # call to reach a given chunk extent on its gather buffer is
                # fully padded with idx-0 descriptors so the buffer region
                # is initialized before any matmul reads it.
                pos = 0
                for g in _groups(int(cb[st, r])):
                    cap = g * P
                    valid = max(0, min(n - pos, cap))
                    if PAD_FULL or g > buf_extent[gi % 4]:
                        buf_extent[gi % 4] = g
                        flat_idx[off + pos + valid : off + pos + cap] = 0
                        valid = cap
                    elif valid == 0:
                        flat_idx[off + pos] = 0
                        valid = 1
                    call_cnt.append(valid)
                    pos += cap
                    gi += 1
                off += int(cb[st, r]) * P

        # gather idx layout: flat slot i -> [i%16, i//16], replicated over
        # the 8 16-partition groups.
        gidx16 = flat_idx.reshape(-1, 16).T  # [16, npad/16]
        gidx = np.ascontiguousarray(np.tile(gidx16, (8, 1)))  # [128, cols]

        # per-slot one-hot targets/values, slot (ch, p) -> [p, ch]
        tgt = np.ascontiguousarray(
            flat_b.reshape(nch, P).T.astype(np.float32)
        )
        vals = np.ascontiguousarray(flat_v.reshape(nch, P).T)

        per_core.append(
            {
                "gidx": gidx,
                "tgt": tgt,
                "val": vals,
                "counts": np.asarray([call_cnt], dtype=np.int32),
            }
        )
    return cbs, per_core


def kernel(
    feature_indices_0,
    feature_values_0,
    feature_indices_1,
    feature_values_1,
    merged_weight,
    bias,
):
    global LAST_RESULT
    idx = np.stack(
        [
            np.asarray(feature_indices_0, dtype=np.int64),
            np.asarray(feature_indices_1, dtype=np.int64),
        ]
    )
    val = np.stack(
        [
            np.asarray(feature_values_0, dtype=np.float32),
            np.asarray(feature_values_1, dtype=np.float32),
        ]
    )
    w = np.ascontiguousarray(np.asarray(merged_weight, dtype=np.float32))
    w8 = w.reshape(Q, ESTEP)
    b = np.asarray(bias, dtype=np.float32)
    bias_rep = np.ascontiguousarray(np.broadcast_to(b[None, :], (P, D)))

    cbs, per_core = _pack_inputs(idx, val)
    nc = _get_nc(cbs)

    cidx = np.ascontiguousarray(
        np.broadcast_to(np.arange(P, dtype=np.float32)[None, :], (P, P))
    )
    in_maps = []
    for c in range(N_CORES):
        in_maps.append(
            {
                "weight": w8,
                "gidx": per_core[c]["gidx"],
                "tgt": per_core[c]["tgt"],
                "val": per_core[c]["val"],
                "cidx": cidx,
                "counts": per_core[c]["counts"],
                "bias_rep": bias_rep,
            }
        )

    res = run_bass_kernel_spmd(
        nc, in_maps, core_ids=list(range(N_CORES)), trace=TRACE
    )
    LAST_RESULT = res
    out0 = np.concatenate([res.results[c]["out0"] for c in range(N_CORES)], axis=0)
    out1 = np.concatenate([res.results[c]["out1"] for c in range(N_CORES)], axis=0)
    return out0, out1
